# revision 8
# baseline (speedup 1.0000x reference)
"""Trainium2 Bass kernel for LorentzSelfAttention (B=8, L=2048, D=128, 1 head).

Sharding: data-parallel over batch — core b handles batch element b.

v3 design (per core, L=2048, D=128, 16 row-chunks of 128):
  Inputs arrive HOST-PACKED per tensor as ONE bf16 dram tensor
  [D, L + D (+16)]: xT | W^T (| pad fat). The W block and first 512-col
  slab ship as separate DMAs so the first proj matmul starts ~2.5us
  earlier; issues are split across the two HWDGE queues (sync / scalar).

  Phase B (projections) in bf16, per-tensor software-pipelined:
    pass1(q), pass1(k), stats2(q), finish3(q), pass1(v), stats2(k),
    finish3(k), stats2(v), finish3(v) — so the PE is never parked
    behind a stats chain (stats2(q) runs on vector while the PE does
    pass1(k) matmuls).
    pass1: 4 proj matmuls / group into PSUM; Sigmoid(col0) + Square
      (scalar) + row-sum (vector). PSUM tiles live until finish3
      (ps_l bufs=6) so the Lorentz scale is applied STRAIGHT from
      PSUM — no park copy.
    stats2: sqrt(s) via Newton rsqrt on the VECTOR engine (bit-trick
      seed + 2 iterations). This avoids the Sqrt ACT table entirely:
      with per-tensor pipelining a scalar Sqrt would thrash the
      sigmoid<->sqrt table sets (~1.3us per switch). Scalar uses only
      Sigmoid (loaded during the input DMA wait) then Exp (one swap).
    finish3: signed time into col 0, narrow scaled from PSUM, q/k
      chunks bf16 -> PE transpose (1-pass) -> qT/kT bf16 (psum->sbuf
      copy on the SCALAR engine); v natural f32r, pad folded into
      its scales.
  Phase C (attention): scores TRANSPOSED S_T[j, i] = <k_j, q_i>_L in
    bf16, exp() without max-subtract/normalize (the final Lorentz
    normalization is scale-invariant so softmax constants cancel),
    exp per 512-col PSUM bank, expT fp32(r), AV outT[d,i] += v_j.T @
    expT_j in f32r (bf16 AV loses too much precision: 9e-2 vs 5.5e-3).
    Software-pipelined: QK_{j+1} emitted before AV_j so the PE chews
    scores while ACT runs exp_j. PSUM: outT 4 banks + 2x1024 slabs.
  Phase D: outT PSUM banks flushed (copy + DMA) as soon as their
    accumulation closes (j = 4b+3), overlapping phase C. The device
    ships the UNNORMALIZED transposed mid-point aveT; the host
    transposes and applies the scale-invariant Lorentz normalization
    out = ave/sqrt(|<ave,ave>_L|) in float64 (exact).

Rows whose allowed (causal & non-pad) key set is empty produce softmax
over an all -inf row in the reference (== uniform over ALL 2048 keys).
Those rows (a ~0-2 row prefix per batch, only when the batch's first
keys are padded) are fixed up exactly on host.
"""

import os

import numpy as np

B, L, D = 8, 2048, 128
P = 128
NCHUNK = L // P   # 16
G = 4             # chunks per group
NGROUP = NCHUNK // G  # 4
XW = L + D        # packed xT|W^T width
XWV = L + D + NCHUNK  # v pack adds pad fat tile

_RUNNER_CACHE: dict = {}


def _bcast3(bass, ap2, inner):
    """[P, n] AP -> [P, n, inner] broadcast view (step-0 innermost)."""
    return bass.AP(tensor=ap2.tensor, offset=ap2.offset,
                   ap=[ap2.ap[0], ap2.ap[1], [0, inner]])


# ---------------------------------------------------------------- device code
def _build_program(cfg, consts):
    from contextlib import ExitStack

    import concourse.bacc as bacc
    import concourse.bass as bass
    import concourse.mybir as mybir
    import concourse.tile as tile
    from concourse import masks

    f32 = mybir.dt.float32
    f32r = mybir.dt.float32r
    bf16 = mybir.dt.bfloat16
    u32 = mybir.dt.uint32
    AF = mybir.ActivationFunctionType
    OP = mybir.AluOpType

    es = {"q": consts["es_q"], "k": consts["es_k"], "v": consts["es_v"]}
    c1 = consts["c1"]
    has_bias = consts["has_bias"]
    sqrt_mode = consts["sqrt_mode"]

    nc = bacc.Bacc("TRN2", target_bir_lowering=False, debug=False)

    xin_d = {
        "q": nc.dram_tensor("xq", [D, XW], bf16, kind="ExternalInput").ap(),
        "k": nc.dram_tensor("xk", [D, XW], bf16, kind="ExternalInput").ap(),
        "v": nc.dram_tensor("xv", [D, XWV], bf16, kind="ExternalInput").ap(),
    }
    bias_d = {}
    if has_bias:
        for nm in ("q", "k", "v"):
            bias_d[nm] = nc.dram_tensor(f"b{nm}", [1, D], f32,
                                        kind="ExternalInput").ap()
    out_d = nc.dram_tensor("out", [D, L], f32, kind="ExternalOutput").ap()

    TENSORS = ("q", "k", "v")

    with tile.TileContext(nc) as tc, ExitStack() as octx:
        cpool = octx.enter_context(tc.tile_pool(name="consts", bufs=1))

        # input DMAs: W + first slab first (critical path), rest after;
        # split across the two HWDGE queues
        xin_sb = {}
        for nm in TENSORS:
            w = XWV if nm == "v" else XW
            xin_sb[nm] = cpool.tile([P, w], bf16, name=f"xin_{nm}",
                                    tag=f"xin_{nm}")
        S0 = G * P  # 512
        nc.sync.dma_start(out=xin_sb["q"][:, L:XW], in_=xin_d["q"][:, L:XW])
        nc.sync.dma_start(out=xin_sb["q"][:, 0:S0], in_=xin_d["q"][:, 0:S0])
        nc.scalar.dma_start(out=xin_sb["k"][:, L:XW], in_=xin_d["k"][:, L:XW])
        nc.scalar.dma_start(out=xin_sb["k"][:, 0:S0], in_=xin_d["k"][:, 0:S0])
        nc.sync.dma_start(out=xin_sb["q"][:, S0:L], in_=xin_d["q"][:, S0:L])
        nc.scalar.dma_start(out=xin_sb["k"][:, S0:L], in_=xin_d["k"][:, S0:L])
        nc.sync.dma_start(out=xin_sb["v"][:], in_=xin_d["v"][:, :])

        ident_bf = cpool.tile([P, P], bf16)
        masks.make_identity(nc, ident_bf[:])
        ut01 = cpool.tile([P, P], f32)
        masks.make_upper_triangular(nc, ut01[:], val=1.0, diag=True)

        bias_sb = {}
        if has_bias:
            for nm in TENSORS:
                bt = cpool.tile([P, D], f32)
                bd = bias_d[nm]
                nc.scalar.dma_start(out=bt[:], in_=bass.AP(
                    tensor=bd.tensor, offset=bd.offset, ap=[[0, P], bd.ap[1]]))
                bias_sb[nm] = bt

        pad_sb = cpool.tile([P, NCHUNK], f32)
        nc.vector.tensor_copy(pad_sb[:], xin_sb["v"][:, L + D:L + D + NCHUNK])

        # persistent activations
        qT_sb = cpool.tile([P, L], bf16)      # [d, l], time row negated
        kT_sb = cpool.tile([P, L], bf16)
        v_sb = cpool.tile([P, NCHUNK, D], f32r)  # [l%128, chunk, d], pad-zeroed
        qk_nat = cpool.tile([P, 2, NCHUNK, D], bf16)  # q/k natural chunks
        outT_sb = cpool.tile([P, L], f32)

        # batched per-row stats: col t*16+c is chunk c of tensor t
        time_all = cpool.tile([P, 3 * NCHUNK], f32)
        ss_all = cpool.tile([P, 3 * NCHUNK], f32)
        sqs_all = cpool.tile([P, 3 * NCHUNK], f32)

        def wview(nm):
            return xin_sb[nm][:, L:L + D]

        def dest4(nm, g):
            if nm == "v":
                return v_sb[:, g * G:(g + 1) * G, :]
            ti = TENSORS.index(nm)
            return qk_nat[:, ti, g * G:(g + 1) * G, :]

        with ExitStack() as ctxB:
            ps_l = ctxB.enter_context(tc.tile_pool(name="ps_l", bufs=6, space="PSUM"))
            ps_q = ctxB.enter_context(tc.tile_pool(name="ps_q", bufs=2, space="PSUM"))
            misc = ctxB.enter_context(tc.tile_pool(name="misc", bufs=3))
            stat = ctxB.enter_context(tc.tile_pool(name="stat", bufs=6))

            lin_ps = {}   # (nm, g) -> live PSUM tile with raw linear output

            def pass1(nm):
                """proj matmuls + sigmoid/sum-sq stats; PSUM stays live."""
                ti = TENSORS.index(nm)
                for g in range(NGROUP):
                    sb = ti * NCHUNK + g * G
                    lin4 = ps_l.tile([P, G, D], f32, tag="lin")
                    for c in range(G):
                        nc.tensor.matmul(
                            lin4[:, c, :],
                            xin_sb[nm][:, (g * G + c) * P:(g * G + c + 1) * P],
                            wview(nm), start=True, stop=True)
                    if has_bias:
                        nc.vector.tensor_add(
                            lin4[:], lin4[:],
                            bass.AP(tensor=bias_sb[nm].tensor,
                                    offset=bias_sb[nm][:].offset,
                                    ap=[bias_sb[nm][:].ap[0], [0, G], [1, D]]))
                    lin_ps[(nm, g)] = lin4
                    nc.scalar.activation(
                        time_all[:, sb:sb + G], lin4[:, :, 0:1], AF.Sigmoid)
                    sq4 = misc.tile([P, G, D - 1], f32, tag="sq4")
                    nc.scalar.activation(sq4[:], lin4[:, :, 1:D], AF.Square)
                    nc.vector.tensor_reduce(
                        ss_all[:, sb:sb + G], sq4[:], mybir.AxisListType.X,
                        OP.add)

            def stats2(nm):
                """batched per-tensor stats on [128, 16] tiles (vector only)."""
                ti = TENSORS.index(nm)
                sb = ti * NCHUNK
                tsl = time_all[:, sb:sb + NCHUNK]
                ssl = ss_all[:, sb:sb + NCHUNK]
                sql = sqs_all[:, sb:sb + NCHUNK]
                # time = sig*e^s + 1.1
                nc.vector.tensor_scalar(
                    out=tsl, in0=tsl, scalar1=es[nm], scalar2=1.1,
                    op0=OP.mult, op1=OP.add)
                inv = stat.tile([P, NCHUNK], f32, tag="inv")
                nc.vector.reciprocal(inv[:], ssl)
                sval = stat.tile([P, NCHUNK], f32, tag="sval")
                nc.vector.tensor_mul(sval[:], tsl, tsl)
                nc.vector.tensor_scalar_add(out=sval[:], in0=sval[:],
                                            scalar1=-1.0)
                nc.vector.tensor_mul(sval[:], sval[:], inv[:])
                if sqrt_mode == "act":
                    nc.scalar.activation(sql, sval[:], AF.Sqrt)
                else:
                    # Newton rsqrt on DVE: seed r0 = bits(0x5f3759df - x>>1),
                    # two iterations r *= 1.5 - 0.5*x*r^2, then sqrt = x*r
                    r = stat.tile([P, NCHUNK], f32, tag="nr")
                    tmp = stat.tile([P, NCHUNK], f32, tag="nt")
                    nc.vector.tensor_scalar(
                        out=r[:].bitcast(u32), in0=sval[:].bitcast(u32),
                        scalar1=1, scalar2=None, op0=OP.logical_shift_right)
                    # 0x5f3759df - t == ~t + 0x5f3759e0 (mod 2^32)
                    nc.vector.tensor_scalar(
                        out=r[:].bitcast(u32), in0=r[:].bitcast(u32),
                        scalar1=0xFFFFFFFF, scalar2=None, op0=OP.bitwise_xor)
                    nc.vector.tensor_scalar(
                        out=r[:].bitcast(u32), in0=r[:].bitcast(u32),
                        scalar1=0x5F3759E0, scalar2=None, op0=OP.add)
                    for _ in range(2):
                        nc.vector.tensor_mul(tmp[:], r[:], r[:])
                        nc.vector.tensor_mul(tmp[:], tmp[:], sval[:])
                        nc.vector.tensor_scalar(
                            out=tmp[:], in0=tmp[:], scalar1=-0.5, scalar2=1.5,
                            op0=OP.mult, op1=OP.add)
                        nc.vector.tensor_mul(r[:], r[:], tmp[:])
                    nc.vector.tensor_mul(sql, sval[:], r[:])
                if nm == "v":   # fold pad zeroing into v scales
                    nc.vector.tensor_mul(sql, sql, pad_sb[:])
                    nc.vector.tensor_mul(tsl, tsl, pad_sb[:])

            def finish3(nm):
                """signed time col, scale narrow straight from PSUM; q/k:
                transpose to qT/kT (psum->sbuf copy on scalar)."""
                ti = TENSORS.index(nm)
                tsign = -1.0 if nm == "q" else 1.0
                dst = qT_sb if nm == "q" else kT_sb
                for g in range(NGROUP):
                    sb = ti * NCHUNK + g * G
                    ch4 = dest4(nm, g)
                    lin4 = lin_ps.pop((nm, g))
                    nc.vector.tensor_scalar(
                        out=ch4[:, :, 0:1], in0=time_all[:, sb:sb + G],
                        scalar1=tsign, scalar2=0.0, op0=OP.mult, op1=OP.add)
                    nc.vector.tensor_mul(
                        ch4[:, :, 1:D], lin4[:, :, 1:D],
                        _bcast3(bass, sqs_all[:, sb:sb + G], D - 1))
                    if nm != "v":
                        qkT4 = ps_q.tile([P, G * P], bf16, tag="qkT")
                        for c in range(G):
                            nc.tensor.transpose(
                                qkT4[:, c * P:(c + 1) * P], ch4[:, c, :],
                                ident_bf[:])
                        nc.scalar.copy(
                            dst[:, g * G * P:(g + 1) * G * P], qkT4[:])

            # per-tensor software pipeline
            pass1("q")
            pass1("k")
            stats2("q")
            finish3("q")
            pass1("v")
            stats2("k")
            finish3("k")
            stats2("v")
            finish3("v")

        # ---------------- Phase C: attention (+ streamed output flush) ----
        with ExitStack() as ctxC:
            ps_s = ctxC.enter_context(tc.tile_pool(name="ps_s", bufs=2, space="PSUM"))
            ps_o = ctxC.enter_context(tc.tile_pool(name="ps_o", bufs=1, space="PSUM"))
            sb_e = ctxC.enter_context(tc.tile_pool(name="sb_e", bufs=2))
            outT_ps = ps_o.tile([P, L], f32)

            exp_tiles = {}

            def emit_qk(j):
                ncols = (NCHUNK - j) * P
                base = j * P
                expT = sb_e.tile([P, L], f32r, tag="expT")
                exp_tiles[j] = expT
                kblk = kT_sb[:, base:base + P]
                ofs = 0
                while ofs < ncols:   # PSUM slabs of <=1024 cols
                    sw = min(1024, ncols - ofs)
                    s_ps = ps_s.tile([P, 1024], f32, tag="s")
                    mofs = 0
                    while mofs < sw:  # matmul + exp per 512-col PSUM bank
                        w = min(512, sw - mofs)
                        o = ofs + mofs
                        nc.tensor.matmul(
                            s_ps[:, mofs:mofs + w], kblk,
                            qT_sb[:, base + o:base + o + w],
                            start=True, stop=True)
                        nc.scalar.activation(
                            expT[:, o:o + w], s_ps[:, mofs:mofs + w],
                            AF.Exp, scale=c1)
                        mofs += w
                    ofs += sw

            def emit_av(j):
                base = j * P
                expT = exp_tiles.pop(j)
                # causal mask inside the diagonal block
                nc.vector.tensor_mul(expT[:, 0:P], expT[:, 0:P], ut01[:])
                # outT[d, i] += v_j.T @ expT_j  (bank-aligned psum chunks)
                col = base
                while col < L:
                    bank_end = min(L, (col // 512 + 1) * 512)
                    kbank = bank_end // 512 - 1
                    last_j = 4 * kbank + 3
                    nc.tensor.matmul(
                        outT_ps[:, col:bank_end],
                        v_sb[:, j, :],
                        expT[:, col - base:bank_end - base],
                        start=(j == 0), stop=(j == last_j))
                    col = bank_end
                # stream out any psum bank whose accumulation just closed
                if j % G == G - 1:
                    b = j // G
                    nc.vector.tensor_copy(
                        outT_sb[:, b * 512:(b + 1) * 512],
                        outT_ps[:, b * 512:(b + 1) * 512])
                    eng = nc.sync if b % 2 == 0 else nc.scalar
                    eng.dma_start(out=out_d[:, b * 512:(b + 1) * 512],
                                  in_=outT_sb[:, b * 512:(b + 1) * 512])

            # software pipeline: QK_{j+1} is on the PE queue before AV_j,
            # so the PE works on next scores while ACT runs exp_j
            emit_qk(0)
            for j in range(NCHUNK):
                if j + 1 < NCHUNK:
                    emit_qk(j + 1)
                emit_av(j)

    nc.compile()
    return nc


def _get_runner(cfg_key, consts):
    if cfg_key in _RUNNER_CACHE:
        return _RUNNER_CACHE[cfg_key]
    nc = _build_program({}, consts)
    _RUNNER_CACHE[cfg_key] = nc
    return nc


# ---------------------------------------------------------------- host logic
def _to_bf16(x):
    import ml_dtypes
    return np.asarray(x, dtype=np.float32).astype(ml_dtypes.bfloat16)


def _host_fixup_rows(out, value, mask, Wv, bv, sv):
    """Exactly reproduce reference for rows with no allowed keys."""
    for b in range(B):
        cnt = np.cumsum(~mask[b])
        rows = np.where(cnt == 0)[0]
        if rows.size == 0:
            continue
        x = value[b].astype(np.float32) @ Wv.T.astype(np.float32) + bv
        time = 1.0 / (1.0 + np.exp(-x[:, :1])) * np.exp(sv) + 1.1
        xn = x[:, 1:]
        s = (time * time - 1.0) / np.sum(xn * xn, axis=-1, keepdims=True)
        vproj = np.concatenate([time, xn * np.sqrt(s)], axis=-1)
        ave = vproj.mean(axis=0)
        lor = -ave[0] ** 2 + np.sum(ave[1:] ** 2)
        denom = np.sqrt(max(abs(lor), 1e-8))
        out[b, rows] = (ave / denom).astype(np.float32)


def kernel(query, key, value, mask, Wq, bq, sq, Wk, bk, sk, Wv, bv, sv,
           attn_scale, attn_bias):
    from concourse.bass_utils import run_bass_kernel_spmd

    query = np.asarray(query, dtype=np.float32)
    key = np.asarray(key, dtype=np.float32)
    value = np.asarray(value, dtype=np.float32)
    mask = np.asarray(mask).astype(bool)
    Wq, Wk, Wv = (np.asarray(w, dtype=np.float32) for w in (Wq, Wk, Wv))
    bq, bk, bv = (np.asarray(b, dtype=np.float32).reshape(-1)
                  for b in (bq, bk, bv))

    has_bias = bool(np.any(bq) or np.any(bk) or np.any(bv))
    consts = dict(
        es_q=float(np.exp(np.float32(sq))),
        es_k=float(np.exp(np.float32(sk))),
        es_v=float(np.exp(np.float32(sv))),
        c1=float(2.0 / np.asarray(attn_scale, dtype=np.float32).reshape(-1)[0]),
        has_bias=has_bias,
        sqrt_mode=os.environ.get("LK_SQRT", "newton"),
    )
    cfg_key = tuple(sorted(consts.items()))
    nc = _get_runner(cfg_key, consts)

    pad01 = (~mask).astype(np.float32)
    # pad fat layout [p, c]: element (p, c) = pad01[c*128 + p]
    pad_fat = pad01.reshape(B, NCHUNK, P).transpose(0, 2, 1)
    wt_bf = {"q": _to_bf16(Wq.T), "k": _to_bf16(Wk.T), "v": _to_bf16(Wv.T)}
    in_maps = []
    for b in range(B):
        m = {
            "xq": np.ascontiguousarray(
                np.concatenate([_to_bf16(query[b].T), wt_bf["q"]], axis=1)),
            "xk": np.ascontiguousarray(
                np.concatenate([_to_bf16(key[b].T), wt_bf["k"]], axis=1)),
            "xv": np.ascontiguousarray(
                np.concatenate([_to_bf16(value[b].T), wt_bf["v"],
                                _to_bf16(pad_fat[b])], axis=1)),
        }
        if has_bias:
            m["bq"] = bq.reshape(1, D)
            m["bk"] = bk.reshape(1, D)
            m["bv"] = bv.reshape(1, D)
        in_maps.append(m)

    res = run_bass_kernel_spmd(nc, in_maps, core_ids=list(range(B)))
    # device ships unnormalized transposed mid-point aveT [D, L];
    # normalization is scale-invariant so it can run on host exactly
    out = np.empty((B, L, D), np.float32)
    for b in range(B):
        ave = np.asarray(res.results[b]["out"]).T.astype(np.float64)
        lor = -ave[:, :1] ** 2 + np.sum(ave[:, 1:] ** 2, axis=-1,
                                        keepdims=True)
        denom = np.sqrt(np.maximum(np.abs(lor), 1e-8))
        out[b] = (ave / denom).astype(np.float32)
    _host_fixup_rows(out, value, mask, Wv, bv, float(np.float32(sv)))
    return out
